# revision 2
# baseline (speedup 1.0000x reference)
"""BitSelfAttention TRN2 kernel (8 NeuronCores, tensor-parallel over heads +
batch-parallel over B).

Sharding: core c -> batch b=c//4, head group hg=c%4 (heads 4*hg..4*hg+3).
Each core: ternary-dequantizes its weight shards on device, computes its 4
heads' Q/K/V projections (fp32r matmuls), RoPE, causal attention with
no-max softmax (scores ~ N(0,1), exp cannot overflow; denominator via a
ones-row matmul), and its partial o_proj (row-parallel).  Host sums the 4
partials per batch.

Self-contained: includes a BIR legalizer for the installed walrus (one
sync-wait / sync-update per instruction).
"""
import json
import numpy as np

# ---------------------------------------------------------------- constants
P = 128
T = 2048
D = 2048
N_HEADS_CORE = 4           # heads per core
HD = 128                   # head dim
TB = 512                   # t-block
NTB = T // TB              # 4
G = D // P                 # 16 contraction chunks
OSH = 512                  # per-core qkv output-column shard
EPS = 1e-8
SCALE = HD ** -0.5
NEG = -1e30

_cached = {}


# ------------------------------------------------------------- BIR legalizer
def _legalize_bir_json(bir_json: bytes) -> bytes:
    """This walrus accepts only ONE sync-wait (and update) per instruction.
    Hoist extras onto same-engine NoOps (engine FIFO keeps semantics)."""
    m = json.loads(bir_json)
    n = [0]

    def nop(engine, waits, updates):
        n[0] += 1
        return {"name": f"I-wfix{n[0]}", "opcode": "NoOp", "engine": engine,
                "ins": [], "outs": [],
                "sync_info": {"on_wait": waits, "on_update": updates}}

    for f in m.get("functions", []):
        for blk in f.get("blocks", []):
            out = []
            for inst in blk.get("instructions", []):
                si = inst.get("sync_info")
                if not si:
                    out.append(inst)
                    continue
                waits = si.get("on_wait") or []
                ups = si.get("on_update") or []
                post = []
                if len(waits) > 1:
                    for w in waits[:-1]:
                        out.append(nop(inst["engine"], [w], []))
                    si["on_wait"] = [waits[-1]]
                if len(ups) > 1:
                    assert inst.get("opcode") not in (
                        "DMACopy", "DMATranspose", "DMAGather",
                        "DMAScatterAdd", "TriggerDma"), inst.get("name")
                    si["on_update"] = [ups[0]]
                    for u in ups[1:]:
                        post.append(nop(inst["engine"], [], [u]))
                out.append(inst)
                out.extend(post)
            blk["instructions"] = out
    return json.dumps(m).encode()


def _install_waitfix():
    import concourse.bass_utils as bu
    if getattr(bu, "_bitattn_waitfix", False):
        return
    bu._bitattn_waitfix = True
    orig = bu.compile_bir_kernel

    def patched(bir_json, tmpdir, neff_name="file.neff"):
        return orig(_legalize_bir_json(bir_json), tmpdir, neff_name)

    bu.compile_bir_kernel = patched
    try:
        import concourse.bass2jax as b2j
        if getattr(b2j, "compile_bir_kernel", None) is orig:
            b2j.compile_bir_kernel = patched
    except ImportError:
        pass


# ---------------------------------------------------------------- bass build
def _build_nc():
    import concourse.bass as bass
    import concourse.mybir as mybir
    import concourse.tile as tile
    from contextlib import ExitStack

    FR = mybir.dt.float32r
    F32 = mybir.dt.float32
    AF = mybir.ActivationFunctionType
    ALU = mybir.AluOpType

    nc = bass.Bass(name="bitattn", trn_type="TRN2")
    # xT declared float32r: same 4-byte payload, lets plain HWDGE DMAs feed
    # fp32r matmuls without a casting pass (PE rounds internally).
    xT_in = nc.dram_tensor("xT", [D, T], FR, kind="ExternalInput")
    wqT_in = nc.dram_tensor("wqT", [D, OSH], F32, kind="ExternalInput")
    wkT_in = nc.dram_tensor("wkT", [D, OSH], F32, kind="ExternalInput")
    wvT_in = nc.dram_tensor("wvT", [D, OSH], F32, kind="ExternalInput")
    woT_in = nc.dram_tensor("woT", [OSH, D], F32, kind="ExternalInput")
    ropeC_in = nc.dram_tensor("ropeC", [P, T], F32, kind="ExternalInput")
    ropeS_in = nc.dram_tensor("ropeS", [P, T], F32, kind="ExternalInput")
    tri_in = nc.dram_tensor("tri", [P, P], F32, kind="ExternalInput")
    outT = nc.dram_tensor("outT", [D, T], F32, kind="ExternalOutput")

    xT_v = xT_in[:].rearrange("(g p) t -> g p t", p=P)
    w_views = {
        "q": wqT_in[:].rearrange("(g p) o -> g p o", p=P),
        "k": wkT_in[:].rearrange("(g p) o -> g p o", p=P),
        "v": wvT_in[:].rearrange("(g p) o -> g p o", p=P),
    }
    woT_v = woT_in[:].rearrange("(fc p) o -> fc p o", p=P)
    outT_v = outT[:].rearrange("(ob p) t -> ob p t", p=P)

    with tile.TileContext(nc) as tc, ExitStack() as ctx:
        dram = ctx.enter_context(tc.tile_pool(name="dram", bufs=1, space="DRAM"))
        qT_d = dram.tile([N_HEADS_CORE, P, T], FR)
        kT_d = dram.tile([N_HEADS_CORE, P, T], FR)
        v_d = dram.tile([N_HEADS_CORE, G, P, HD], FR)
        srow_d = dram.tile([3 * G + G, 2, OSH], F32)   # dequant bcast rows
        bc_d = dram.tile([N_HEADS_CORE * NTB, TB], F32)  # denom bcast rows

        glob = ctx.enter_context(tc.tile_pool(name="glob", bufs=1))
        ones_f = glob.tile([P, 1], F32)
        nc.gpsimd.memset(ones_f[:], 1.0)
        ones_r = glob.tile([P, 1], FR)
        nc.vector.tensor_copy(ones_r[:], ones_f[:])
        bias_m1 = glob.tile([P, 1], F32)
        nc.gpsimd.memset(bias_m1[:], -1.0)
        bias_p1 = glob.tile([P, 1], F32)
        nc.gpsimd.memset(bias_p1[:], 1.0)
        onesrow_f = glob.tile([1, P], F32)
        nc.gpsimd.memset(onesrow_f[:], 1.0)
        onesrow_r = glob.tile([1, P], FR)
        nc.vector.tensor_copy(onesrow_r[:], onesrow_f[:])

        # ---------------- dequant chunk helper -------------------------
        # q*scale = Sign(w) * (2|w| > s) * s, with s = max(sum(2|w|)/256, EPS).
        # No division/reciprocal, one broadcast row, exact at thresholds.
        def dequant_chunk(dq, psum, src_ap, dst_ap, row_idx,
                          ssum_tag="ssum", ssum_bufs=1):
            wt = dq.tile([P, OSH], F32, tag="wt", name="wt")
            nc.sync.dma_start(wt[:], src_ap)
            ab2 = dq.tile([P, OSH], F32, tag="ab2", name="ab2")
            nc.scalar.activation(ab2[:], wt[:], AF.Abs, scale=2.0)
            ssum = psum.tile([1, OSH], F32, tag=ssum_tag, name="ssum",
                             bufs=ssum_bufs)
            nc.tensor.matmul(ssum[:], ones_f[:], ab2[:], start=True, stop=True)
            srow = dq.tile([1, OSH], F32, tag="srow", name="srow", bufs=2)
            nc.vector.tensor_scalar(srow[:], ssum[:], 1.0 / 256.0, EPS,
                                    ALU.mult, ALU.max)
            nc.sync.dma_start(srow_d[row_idx, 0:1], srow[:])
            sb_ = dq.tile([P, OSH], F32, tag="sb_", name="sb_")
            nc.sync.dma_start(sb_[:], srow_d[row_idx, 0:1].to_broadcast((P, OSH)))
            sgn = dq.tile([P, OSH], F32, tag="sgn", name="sgn", bufs=2)
            nc.scalar.activation(sgn[:], wt[:], AF.Sign)
            d_ = dq.tile([P, OSH], F32, tag="d_", name="d_", bufs=2)
            nc.vector.tensor_tensor(d_[:], ab2[:], sb_[:], ALU.subtract)
            sd = dq.tile([P, OSH], F32, tag="sd", name="sd", bufs=2)
            nc.scalar.activation(sd[:], d_[:], AF.Sign)
            m = dq.tile([P, OSH], F32, tag="m", name="m", bufs=2)
            nc.vector.tensor_scalar_max(m[:], sd[:], 0.0)
            ms = dq.tile([P, OSH], F32, tag="ms", name="ms", bufs=2)
            nc.vector.tensor_mul(ms[:], m[:], sb_[:])
            nc.vector.tensor_mul(dst_ap, sgn[:], ms[:])

        # ================= phase A+B: dequant + QKV ====================
        with ExitStack() as pctx:
            wpool = pctx.enter_context(tc.tile_pool(name="wdqp", bufs=1))
            w_dq = {
                "q": wpool.tile([P, G, OSH], FR, name="wq_dq"),
                "k": wpool.tile([P, G, OSH], FR, name="wk_dq"),
                "v": wpool.tile([P, G, OSH], FR, name="wv_dq"),
            }
            dq = pctx.enter_context(tc.tile_pool(name="dq", bufs=4))
            psA = pctx.enter_context(tc.tile_pool(name="psA", bufs=2, space="PSUM"))
            for pi, pr in enumerate(("q", "k", "v")):
                for g in range(G):
                    dequant_chunk(dq, psA, w_views[pr][g], w_dq[pr][:, g],
                                  pi * G + g, ssum_bufs=2)

            ropep = pctx.enter_context(tc.tile_pool(name="ropep", bufs=2))

            xpool = pctx.enter_context(tc.tile_pool(name="xp", bufs=1))
            evac = pctx.enter_context(tc.tile_pool(name="evac", bufs=2))
            psQK = pctx.enter_context(tc.tile_pool(name="psQK", bufs=4, space="PSUM"))
            psV = pctx.enter_context(tc.tile_pool(name="psV", bufs=2, space="PSUM"))

            for tb in range(NTB):
                ts = slice(tb * TB, (tb + 1) * TB)
                xTr = xpool.tile([P, G, TB], FR, tag="xTr", name="xTr")
                for g in range(G):
                    nc.sync.dma_start(xTr[:, g], xT_v[g, :, ts])
                ropeC = ropep.tile([P, TB], F32, tag="rc", name="rc", bufs=1)
                nc.sync.dma_start(ropeC[:], ropeC_in[:, ts])
                ropeS = ropep.tile([P, TB], F32, tag="rs", name="rs", bufs=1)
                nc.sync.dma_start(ropeS[:], ropeS_in[:, ts])

                for pr, dst in (("q", qT_d), ("k", kT_d)):
                    for h in range(N_HEADS_CORE):
                        pq = psQK.tile([P, TB], F32, tag="pqk", name="pqk")
                        for g in range(G):
                            nc.tensor.matmul(
                                pq[:], w_dq[pr][:, g, h * HD:(h + 1) * HD],
                                xTr[:, g], start=(g == 0), stop=(g == G - 1))
                        # rope evac: out = q*C + swap(q*S'') with
                        # S'' = [sin; -sin]; swap via sbuf->sbuf DMA
                        qa = evac.tile([P, TB], F32, tag="qa", name="qa", bufs=1)
                        tmp = evac.tile([P, TB], F32, tag="tmp", name="tmp", bufs=1)
                        tsw = evac.tile([P, TB], F32, tag="tsw", name="tsw")
                        qr = evac.tile([P, TB], FR, tag="qr", name="qr")
                        nc.vector.tensor_mul(qa[:], pq[:], ropeC[:])
                        nc.vector.tensor_mul(tmp[:], pq[:], ropeS[:])
                        nc.sync.dma_start(tsw[0:64], tmp[64:128])
                        nc.sync.dma_start(tsw[64:128], tmp[0:64])
                        nc.vector.tensor_add(qr[:], qa[:], tsw[:])
                        nc.sync.dma_start(dst[h, :, ts], qr[:])

                for tk in range(NTB):
                    pv = psV.tile([P, TB], F32, tag="pv", name="pv")
                    for g in range(G):
                        nc.tensor.matmul(
                            pv[:], xTr[:, g, tk * HD:(tk + 1) * HD],
                            w_dq["v"][:, g], start=(g == 0), stop=(g == G - 1))
                    vv = evac.tile([P, TB], FR, tag="vv", name="vv")
                    nc.scalar.copy(vv[:], pv[:])
                    for h in range(N_HEADS_CORE):
                        nc.sync.dma_start(v_d[h, tb * NTB + tk],
                                          vv[:, h * HD:(h + 1) * HD])

        # ============ phase C+D: attention (j-outer) + o_proj ==========
        ypool = ctx.enter_context(tc.tile_pool(name="ypool", bufs=1))
        y_sb = ypool.tile([P, N_HEADS_CORE, T], FR, name="y_sb")

        with ExitStack() as pctx:
            # wo dequant first: independent work that fills the phase-switch
            # pipeline bubble.
            apool0 = pctx.enter_context(tc.tile_pool(name="apool0", bufs=1))
            khs = apool0.tile([P, N_HEADS_CORE, T], FR, tag="khs", name="khs")
            vhs = apool0.tile([P, N_HEADS_CORE, G, HD], FR, tag="vhs", name="vhs")
            wopool = pctx.enter_context(tc.tile_pool(name="wop", bufs=1))
            wo_dq = wopool.tile([P, N_HEADS_CORE, D], FR, name="wo_dq")
            dqD = pctx.enter_context(tc.tile_pool(name="dqD", bufs=2))
            psD = pctx.enter_context(tc.tile_pool(name="psD", bufs=2, space="PSUM"))
            psY = pctx.enter_context(tc.tile_pool(name="psY", bufs=2, space="PSUM"))
            for fc in range(N_HEADS_CORE):
                for oc in range(4):
                    dequant_chunk(dqD, psD,
                                  woT_v[fc, :, oc * OSH:(oc + 1) * OSH],
                                  wo_dq[:, fc, oc * OSH:(oc + 1) * OSH],
                                  3 * G + fc * 4 + oc)

            apool = pctx.enter_context(tc.tile_pool(name="apool", bufs=3))
            expool = pctx.enter_context(tc.tile_pool(name="expool", bufs=4))
            tripool = pctx.enter_context(tc.tile_pool(name="tripool", bufs=1))
            tri = tripool.tile([P, P], F32, name="tri_sb")
            nc.sync.dma_start(tri[:], tri_in[:])
            opool = pctx.enter_context(tc.tile_pool(name="op", bufs=4))
            psS = pctx.enter_context(tc.tile_pool(name="psS", bufs=4, space="PSUM"))

            def oproj_tb(tb):
                ts = slice(tb * TB, (tb + 1) * TB)
                for ob in range(G):
                    ps_o = psS.tile([P, TB], F32, tag="ps", name="ps_o")
                    for fc in range(N_HEADS_CORE):
                        nc.tensor.matmul(
                            ps_o[:], wo_dq[:, fc, ob * P:(ob + 1) * P],
                            y_sb[:, fc, ts],
                            start=(fc == 0), stop=(fc == N_HEADS_CORE - 1))
                    ot = opool.tile([P, TB], F32, tag="ot", name="ot")
                    nc.scalar.copy(ot[:], ps_o[:])
                    nc.sync.dma_start(outT_v[ob, :, ts], ot[:])

            for j in range(NTB):
                nkk = 4 * j + 4
                for h in range(N_HEADS_CORE):
                    if True:
                        nc.sync.dma_start(
                            khs[:, h, 4 * j * P:nkk * P],
                            kT_d[h, :, 4 * j * P:nkk * P])
                        nc.sync.dma_start(
                            vhs[:, h, 4 * j:nkk],
                            v_d[h, 4 * j:nkk].rearrange("k p d -> p k d"))
                for h in range(N_HEADS_CORE):
                    qj = apool.tile([P, TB], FR, tag="qj", name="qj", bufs=4)
                    nc.sync.dma_start(qj[:], qT_d[h, :, j * TB:(j + 1) * TB])

                    ps_y = psY.tile([P, TB], F32, tag="py", name="py")
                    ps_den = psD.tile([1, TB], F32, tag="pd", name="pd", bufs=1)
                    for kk in range(nkk):
                        d = kk - 4 * j
                        off = 128 * d if d >= 0 else 0
                        ncols = TB - off
                        ps_st = psS.tile([P, TB], F32, tag="ps", name="ps")
                        st = ps_st[:, 0:ncols]
                        nc.tensor.matmul(
                            st, khs[:, h, kk * P:(kk + 1) * P],
                            qj[:, off:TB],
                            start=True, stop=True)
                        if d >= 0:
                            nc.vector.tensor_add(ps_st[:, 0:P],
                                                 ps_st[:, 0:P], tri[:])
                        ex = expool.tile([P, TB], FR, tag="ex", name="ex", bufs=6)
                        nc.scalar.activation(ex[:, 0:ncols], st, AF.Exp,
                                             scale=SCALE)
                        nc.tensor.matmul(ps_den[:, off:], ones_r[:],
                                         ex[:, 0:ncols],
                                         start=(kk == 0), stop=(kk == nkk - 1))
                        nc.tensor.matmul(ps_y[:, off:], vhs[:, h, kk],
                                         ex[:, 0:ncols],
                                         start=(kk == 0), stop=(kk == nkk - 1))
                    rec = apool.tile([1, TB], FR, tag="rec", name="rec")
                    with nc.allow_low_precision("f32r bcast of 1/denom"):
                        nc.vector.reciprocal(rec[:], ps_den[:])
                    den_b = psS.tile([P, TB], F32, tag="ps", name="den_b")
                    nc.tensor.matmul(den_b[:], onesrow_r[:], rec[:],
                                     start=True, stop=True)
                    den_s = apool.tile([P, TB], F32, tag="den_s", name="den_s")
                    nc.scalar.copy(den_s[:], den_b[:])
                    nc.vector.tensor_mul(y_sb[:, h, j * TB:(j + 1) * TB],
                                         ps_y[:], den_s[:])
                # all heads of t-block j done -> its o_proj can start
                oproj_tb(j)

    return nc


def _rope_tables():
    half = HD // 2
    inv_freq = 1.0 / (10000.0 ** (np.arange(half, dtype=np.float64) / half))
    freqs = np.outer(np.arange(T, dtype=np.float64), inv_freq)  # [T, 64]
    c = np.cos(freqs).astype(np.float32).T                      # [64, T]
    s = np.sin(freqs).astype(np.float32).T
    # S'' = [sin; -sin]: rope computed as q*C + swap_halves(q*S'')
    return (np.ascontiguousarray(np.concatenate([c, c], axis=0)),
            np.ascontiguousarray(np.concatenate([s, -s], axis=0)))


def _prepare_in_maps(x, w_q, w_k, w_v, w_o):
    x = np.asarray(x, dtype=np.float32)
    w_q = np.asarray(w_q, dtype=np.float32)
    w_k = np.asarray(w_k, dtype=np.float32)
    w_v = np.asarray(w_v, dtype=np.float32)
    w_o = np.asarray(w_o, dtype=np.float32)

    ropeC, ropeS = _rope_tables()
    idx = np.arange(P)
    tri = np.where(idx[:, None] > idx[None, :], np.float32(NEG),
                   np.float32(0.0)).astype(np.float32)

    in_maps = []
    for c in range(8):
        b, hg = divmod(c, 4)
        osl = slice(hg * OSH, (hg + 1) * OSH)
        in_maps.append({
            "xT": np.ascontiguousarray(x[b].T),
            "wqT": np.ascontiguousarray(w_q[osl, :].T),
            "wkT": np.ascontiguousarray(w_k[osl, :].T),
            "wvT": np.ascontiguousarray(w_v[osl, :].T),
            "woT": np.ascontiguousarray(w_o[:, osl].T),
            "ropeC": ropeC, "ropeS": ropeS, "tri": tri,
        })
    return in_maps


def kernel(x, w_q, w_k, w_v, w_o):
    _install_waitfix()
    from concourse.bass_utils import run_bass_kernel_spmd

    B = np.asarray(x).shape[0]

    if "nc" not in _cached:
        _cached["nc"] = _build_nc()
    nc = _cached["nc"]

    in_maps = _prepare_in_maps(x, w_q, w_k, w_v, w_o)

    import os as _os
    trace = _os.environ.get("BITATTN_TRACE") == "1"
    res = run_bass_kernel_spmd(nc, in_maps, core_ids=list(range(8)),
                               trace=trace)
    _cached["last_res"] = res
    out = np.zeros((B, T, D), dtype=np.float32)
    for c in range(8):
        b = c // 4
        out[b] += res.results[c]["outT"].T
    return out

